# revision 5
# baseline (speedup 1.0000x reference)
"""Trainium2 Bass kernel for nn_Attention_11433202942207.

Spatial-reduction attention (PVT-style) on [B=8, N=4096, C=512]:
  q = x @ q_w.T + q_b                          (heads=8, d=64)
  x_sr = LN(conv2x2s2(x) + sr_b) * g + b      (N2=1024)
  k, v = x_sr @ kv_w.T + kv_b
  out = softmax(q k^T / sqrt(d)) v @ proj_w.T + proj_b

Distribution: data-parallel over batch, one batch element per NeuronCore
(8 cores). No collectives needed.

Device strategy (per core, all matmuls float32r = tf32-like @ full PE rate):
  - host pre-transposes x to xT [C, N] with tokens sigma-permuted so the
    2x2/stride-2 conv patches become single-stride access patterns.
  - qT = q_w_scaled @ xT (+b) kept transposed [C, N] in SBUF.
  - conv as matmul over K=(pixel, cin)=2048 with strided lhsT views of xT;
    LN in natural layout; transpose to x_srT via TensorE.
  - kT = kv_w_k @ x_srT (transposed), v natural [N2, C] with per-head
    ones-augmented columns for softmax-denominator fusion.
  - scoresT[key, tok] per head via K=64 matmuls; exp on ScalarE (no max
    subtraction -- logits are O(1) by construction); unnormalized AV with
    the ones column producing the denominator in the same PSUM tile;
    normalization via reciprocal + K=1 broadcast matmul + one multiply.
  - proj consumes attention output in [d, head, tok] layout with K=64
    matmuls; bias add; contiguous store (host un-permutes tokens).
"""

import sys

sys.path.insert(0, "/opt/trn_rl_repo")

import numpy as np

import concourse.bass as bass
from concourse import bacc, mybir
from concourse.tile import TileContext
from concourse.masks import make_identity

F32 = mybir.dt.float32
F32R = mybir.dt.float32r

B, N, C = 8, 4096, 512
NH, D = 8, 64
N2 = 1024
TB = 8          # token blocks of 512
NCORES = 8
LN_EPS = 1e-5


def round_f32r(a: np.ndarray) -> np.ndarray:
    """Round fp32 to fp32r (8-bit exponent, 11-bit mantissa) RNE."""
    u = np.ascontiguousarray(a, dtype=np.float32).view(np.uint32)
    r = (u + 0x7FF + ((u >> 12) & 1)) & np.uint32(0xFFFFF000)
    return r.view(np.float32)


def _sigma_permute(x):
    """[B, 4096, C] row-major tokens -> 2x2-block-interleaved token order."""
    b = x.shape[0]
    return (
        x.reshape(b, 32, 2, 32, 2, C)
        .transpose(0, 1, 3, 2, 4, 5)
        .reshape(b, N, C)
    )


def _sigma_unpermute(y):
    b = y.shape[0]
    return (
        y.reshape(b, 32, 32, 2, 2, C)
        .transpose(0, 1, 3, 2, 4, 5)
        .reshape(b, N, C)
    )


def build_nc() -> bass.Bass:
    nc = bacc.Bacc(target_bir_lowering=False)

    xT = nc.declare_dram_parameter("xT", [C, N], F32R, isOutput=False)
    qw = nc.declare_dram_parameter("q_wT", [C, C], F32R, isOutput=False)
    qb = nc.declare_dram_parameter("q_b", [C], F32, isOutput=False)
    srw = nc.declare_dram_parameter("srw", [4 * C, C], F32R, isOutput=False)
    srb = nc.declare_dram_parameter("sr_b", [C], F32, isOutput=False)
    kvw = nc.declare_dram_parameter("kv_wT", [C, 2 * C], F32R, isOutput=False)
    kvbk = nc.declare_dram_parameter("kv_bk", [C], F32, isOutput=False)
    kvbv = nc.declare_dram_parameter("kv_bv", [C], F32, isOutput=False)
    pw = nc.declare_dram_parameter("proj_wT", [C, C], F32R, isOutput=False)
    pb = nc.declare_dram_parameter("proj_b", [C], F32, isOutput=False)
    out = nc.declare_dram_parameter("out", [N, C], F32, isOutput=True)

    def bcast_load(dst, src_handle):
        ap = src_handle[:]
        nc.gpsimd.dma_start(
            out=dst,
            in_=bass.AP(tensor=ap.tensor, offset=ap.offset, ap=[[0, 128], [1, C]]),
        )

    with nc.allow_low_precision(reason="f32r matmul inputs; accumulation is fp32"):
        with TileContext(nc) as tc:
            # ---- persistent tiles --------------------------------------
            persist_cm = tc.tile_pool(name="persist", bufs=1)
            persist = persist_cm.__enter__()
            qT = persist.tile([128, 4, N], F32R)          # 64KB/part
            x_srT = persist.tile([128, 4, N2], F32R)      # 16KB/part
            kT = persist.tile([128, 4, N2], F32R)         # 16KB/part
            v_sb = persist.tile([128, 8, NH, 66], F32R)   # ~16.5KB/part
            pw_sb = persist.tile([64, NH, C], F32R)       # 16KB/part (64p)
            pb_bc = persist.tile([128, C], F32)
            srb_bc = persist.tile([128, C], F32)
            kvbv_bc = persist.tile([128, C], F32)
            qb_sb = persist.tile([128, 4], F32)
            kvbk_sb = persist.tile([128, 4], F32)
            eps_sb = persist.tile([128, 1], F32)
            ones_col = persist.tile([128, D], F32R)

            nc.vector.memset(eps_sb[:], LN_EPS)
            nc.vector.memset(ones_col[:].bitcast(F32), 1.0)
            nc.vector.memset(v_sb[:, :, :, 64:66].bitcast(F32), 1.0)
            bcast_load(pb_bc[:], pb)
            bcast_load(srb_bc[:], srb)
            bcast_load(kvbv_bc[:], kvbv)
            nc.sync.dma_start(out=qb_sb[:], in_=qb[:].rearrange("(c p) -> p c", p=128))
            nc.sync.dma_start(
                out=kvbk_sb[:], in_=kvbk[:].rearrange("(c p) -> p c", p=128)
            )
            nc.sync.dma_start(
                out=pw_sb[:], in_=pw[:, :].rearrange("(h p) n -> p h n", p=64)
            )

            # ---- phase A: qT, conv+LN -> x_srT -------------------------
            with tc.tile_pool(name="phA", bufs=1) as pa, \
                 tc.tile_pool(name="phA2", bufs=2) as pa2, \
                 tc.tile_pool(name="psA", bufs=2, space="PSUM") as psA:
                ident = pa.tile([128, 128], F32)
                make_identity(nc, ident[:])
                qw_sb = pa.tile([128, 4, C], F32R)
                srw_sb = pa.tile([128, 16, C], F32R)
                nc.sync.dma_start(
                    out=qw_sb[:], in_=qw[:, :].rearrange("(c p) n -> p c n", p=128)
                )
                nc.sync.dma_start(
                    out=srw_sb[:], in_=srw[:, :].rearrange("(k p) n -> p k n", p=128)
                )

                for tb in range(TB):
                    ts = slice(512 * tb, 512 * (tb + 1))
                    xt_tb = pa2.tile([128, 4, 512], F32R, tag="xt")
                    nc.sync.dma_start(
                        out=xt_tb[:],
                        in_=xT[:, :].rearrange("(c p) t -> p c t", p=128)[:, :, ts],
                    )

                    # qT[:, :, ts]
                    for mq in range(4):
                        pq = psA.tile([128, 512], F32, tag="pq")
                        for kc in range(4):
                            nc.tensor.matmul(
                                pq[:],
                                qw_sb[:, kc, 128 * mq:128 * (mq + 1)],
                                xt_tb[:, kc, :],
                                start=(kc == 0),
                                stop=(kc == 3),
                            )
                        nc.vector.tensor_scalar_add(
                            out=qT[:, mq, ts], in0=pq[:], scalar1=qb_sb[:, mq:mq + 1]
                        )

                    # conv chunk -> x_sr natural [128 n2, C]
                    pxsr = psA.tile([128, 512], F32, tag="pxsr")
                    for kc in range(16):
                        p, cb = kc // 4, kc % 4
                        lhs = xt_tb[:, cb, :]
                        lhs = bass.AP(
                            tensor=lhs.tensor, offset=lhs.offset + p, ap=[lhs.ap[0], [4, 128]]
                        )
                        nc.tensor.matmul(
                            pxsr[:],
                            lhs,
                            srw_sb[:, p * 4 + cb, :],
                            start=(kc == 0),
                            stop=(kc == 15),
                        )

                    xsr = pa2.tile([128, 512], F32, tag="xsr")
                    nc.vector.tensor_add(xsr[:], pxsr[:], srb_bc[:, :])
                    stats = pa2.tile([128, 6], F32, tag="stats")
                    nc.vector.bn_stats(out=stats[:], in_=xsr[:])
                    mv = pa2.tile([128, 2], F32, tag="mv")
                    nc.vector.bn_aggr(out=mv[:], in_=stats[:])
                    rstd = pa2.tile([128, 1], F32, tag="rstd")
                    nc.scalar.activation(
                        out=rstd[:],
                        in_=mv[:, 1:2],
                        func=mybir.ActivationFunctionType.Sqrt,
                        bias=eps_sb[:],
                        scale=1.0,
                    )
                    nc.vector.reciprocal(rstd[:], rstd[:])
                    xsrn = pa2.tile([128, 512], F32, tag="xsrn")
                    nc.vector.tensor_scalar(
                        out=xsrn[:],
                        in0=xsr[:],
                        scalar1=mv[:, 0:1],
                        scalar2=rstd[:],
                        op0=mybir.AluOpType.subtract,
                        op1=mybir.AluOpType.mult,
                    )
                    for cb in range(4):
                        ptr = psA.tile([128, 128], F32, tag="ptr")
                        nc.tensor.transpose(
                            ptr[:], xsrn[:, 128 * cb:128 * (cb + 1)], ident[:]
                        )
                        nc.vector.tensor_copy(
                            x_srT[:, cb, 128 * tb:128 * (tb + 1)], ptr[:]
                        )

            # ---- phase B: kT, v ---------------------------------------
            with tc.tile_pool(name="phB", bufs=1) as pbp, \
                 tc.tile_pool(name="psB", bufs=2, space="PSUM") as psB:
                kvw_sb = pbp.tile([128, 4, 2 * C], F32R)
                nc.sync.dma_start(
                    out=kvw_sb[:], in_=kvw[:, :].rearrange("(c p) n -> p c n", p=128)
                )
                for mk in range(4):
                    for nk in range(2):
                        pk = psB.tile([128, 512], F32, tag="pk")
                        for kc in range(4):
                            nc.tensor.matmul(
                                pk[:],
                                kvw_sb[:, kc, 128 * mk:128 * (mk + 1)],
                                x_srT[:, kc, 512 * nk:512 * (nk + 1)],
                                start=(kc == 0),
                                stop=(kc == 3),
                            )
                        nc.vector.tensor_scalar_add(
                            out=kT[:, mk, 512 * nk:512 * (nk + 1)],
                            in0=pk[:],
                            scalar1=kvbk_sb[:, mk:mk + 1],
                        )
                for mv_ in range(8):
                    pv = psB.tile([128, 512], F32, tag="pv")
                    for kc in range(4):
                        nc.tensor.matmul(
                            pv[:],
                            x_srT[:, kc, 128 * mv_:128 * (mv_ + 1)],
                            kvw_sb[:, kc, C:2 * C],
                            start=(kc == 0),
                            stop=(kc == 3),
                        )
                    nc.vector.tensor_add(
                        out=v_sb[:, mv_, :, 0:64],
                        in0=pv[:].rearrange("p (h d) -> p h d", h=NH),
                        in1=kvbv_bc[:, :].rearrange("p (h d) -> p h d", h=NH),
                    )

            # ---- phase C: attention + proj ----------------------------
            with tc.tile_pool(name="phC", bufs=2) as pc, \
                 tc.tile_pool(name="phC3", bufs=3) as pc3, \
                 tc.tile_pool(name="psS", bufs=2, space="PSUM") as psS, \
                 tc.tile_pool(name="psAV", bufs=2, space="PSUM") as psAV, \
                 tc.tile_pool(name="psO", bufs=1, space="PSUM") as psO:
                for tb in range(TB):
                    ts = slice(512 * tb, 512 * (tb + 1))
                    aoT = pc.tile([64, NH, 512], F32R, tag="aoT")
                    for h in range(NH):
                        hb = 64 * (h % 2)
                        hc = h // 2
                        pav = psAV.tile([65, 512], F32, tag="pav")
                        for j in range(4):
                            ps_ = psS.tile([128, 1024], F32, tag="ps_s")
                            for u in range(2):
                                kc = 2 * j + u
                                nc.tensor.matmul(
                                    ps_[:, 512 * u:512 * (u + 1)],
                                    kT[hb:hb + 64, hc, 128 * kc:128 * (kc + 1)],
                                    qT[hb:hb + 64, hc, ts],
                                    start=True,
                                    stop=True,
                                )
                            expT = pc3.tile([128, 1024], F32R, tag="expT")
                            nc.scalar.activation(
                                out=expT[:], in_=ps_[:],
                                func=mybir.ActivationFunctionType.Exp,
                            )
                            for u in range(2):
                                kc = 2 * j + u
                                nc.tensor.matmul(
                                    pav[:],
                                    v_sb[:, kc, h, 0:65],
                                    expT[:, 512 * u:512 * (u + 1)],
                                    start=(kc == 0),
                                    stop=(kc == 7),
                                )
                        # normalize: recip of ones-row, broadcast, multiply
                        recs = pc.tile([128, 512], F32R, tag="recs")
                        nc.vector.reciprocal(recs[64:65, :], pav[64:65, :])
                        pbc = psO.tile([64, 512], F32, tag="pbc")
                        nc.tensor.matmul(
                            pbc[:], ones_col[64:65, :], recs[64:65, :],
                            start=True, stop=True,
                        )
                        bc_sb = pc.tile([64, 512], F32, tag="bc_sb")
                        nc.vector.tensor_copy(bc_sb[:], pbc[:])
                        nc.vector.tensor_mul(aoT[:, h, :], pav[0:64, :], bc_sb[:])

                    # proj for this token block
                    for mo in range(4):
                        po = psO.tile([128, 512], F32, tag="po")
                        for h in range(NH):
                            nc.tensor.matmul(
                                po[:],
                                aoT[:, h, 128 * mo:128 * (mo + 1)],
                                pw_sb[:, h, :],
                                start=(h == 0),
                                stop=(h == NH - 1),
                            )
                        osb = pc.tile([128, 512], F32, tag="osb")
                        nc.vector.tensor_add(osb[:], po[:], pb_bc[:, :])
                        nc.sync.dma_start(
                            out=out[512 * tb + 128 * mo:512 * tb + 128 * (mo + 1), :],
                            in_=osb[:],
                        )

            persist_cm.__exit__(None, None, None)

    nc.compile()
    return nc


def prep_in_maps(x, q_w, q_b, kv_w, kv_b, sr_w, sr_b, ln_g, ln_b, proj_w, proj_b):
    x = np.asarray(x, np.float32)
    q_w = np.asarray(q_w, np.float32)
    q_b = np.asarray(q_b, np.float32)
    kv_w = np.asarray(kv_w, np.float32)
    kv_b = np.asarray(kv_b, np.float32)
    sr_w = np.asarray(sr_w, np.float32)
    sr_b = np.asarray(sr_b, np.float32)
    ln_g = np.asarray(ln_g, np.float32)
    ln_b = np.asarray(ln_b, np.float32)
    proj_w = np.asarray(proj_w, np.float32)
    proj_b = np.asarray(proj_b, np.float32)

    scale = float(D) ** -0.5
    xT = round_f32r(_sigma_permute(x).transpose(0, 2, 1))        # [B, C, N]
    q_wT = round_f32r((q_w * scale).T)                           # [C, C]
    q_bs = (q_b * scale).astype(np.float32)
    srw = round_f32r(np.transpose(sr_w, (2, 3, 1, 0)).reshape(4 * C, C))
    kv_w_eff = kv_w * ln_g[None, :]
    kv_b_eff = (kv_b + kv_w @ ln_b).astype(np.float32)
    kv_wT = round_f32r(kv_w_eff.T)                               # [C, 2C]
    proj_wT = round_f32r(proj_w.T)

    shared = {
        "q_wT": q_wT, "q_b": q_bs, "srw": srw, "sr_b": sr_b,
        "kv_wT": kv_wT, "kv_bk": kv_b_eff[:C], "kv_bv": kv_b_eff[C:],
        "proj_wT": proj_wT, "proj_b": proj_b,
    }
    return [dict(shared, xT=np.ascontiguousarray(xT[i])) for i in range(NCORES)]


_CACHED = {}


def _get_nc():
    if "nc" not in _CACHED:
        _CACHED["nc"] = build_nc()
    return _CACHED["nc"]


def kernel(x, q_w, q_b, kv_w, kv_b, sr_w, sr_b, ln_g, ln_b, proj_w, proj_b,
           H=64, W=64):
    from concourse.bass_utils import run_bass_kernel_spmd

    nc = _get_nc()
    in_maps = prep_in_maps(x, q_w, q_b, kv_w, kv_b, sr_w, sr_b, ln_g, ln_b,
                           proj_w, proj_b)
    res = run_bass_kernel_spmd(nc, in_maps, list(range(NCORES)), trace=False)
    out_perm = np.stack([res.results[i]["out"] for i in range(NCORES)], axis=0)
    return _sigma_unpermute(out_perm).astype(np.float32)
